# revision 9
# baseline (speedup 1.0000x reference)
"""Trainium2 Bass kernel for nn_LinearQuantizerModel.

MLP 1024->894->763->501 (leaky_relu 0.01) + argmax over classes + exact
forward-fill of stop tokens (==500) done on host.

Sharding: data-parallel over batch B=16 across 8 cores (2 batches/core),
weights replicated. Per core 4000 tokens padded to 4096 = 8 chunks x 512.

Device layout: features on partitions, tokens on free axis. x is
transposed on host so DMA loads are contiguous. Matmuls run in fp32r
(full-rate PE). Layer-3 flips orientation (stationary = H2T token tile,
moving = W3) so logits land [tokens, 501] in PSUM for vector argmax.
"""

import numpy as np

import concourse.bass as bass
import concourse.mybir as mybir
import concourse.tile as tile
from concourse import bacc
from concourse.bass_utils import run_bass_kernel_spmd

B, T, DIM, H1, H2, OUT = 16, 2000, 1024, 894, 763, 501
OUTP = 512  # class dim padded for fp32r ISA (even/aligned free dim)
VOCAB = 500
MAX_ITERS = 10000
NCORES = 8
TOK = 4096          # padded tokens per core (4000 real)
REAL_TOK = 4000
NCHUNK = 8
CH = 512            # tokens per chunk
NSUB = TOK // 128   # 32 code columns

F32 = mybir.dt.float32
F32R = mybir.dt.float32r
MM_DT = F32R        # fp32r: full-rate PE, ~1e-4 matmul precision

_CACHE = {}


def _ceil(a, b):
    return (a + b - 1) // b


def build_kernel():
    nc = bacc.Bacc(target_bir_lowering=False)

    xT = nc.dram_tensor("xT", [DIM, TOK], MM_DT, kind="ExternalInput")
    W1d = nc.dram_tensor("W1", [DIM, H1], MM_DT, kind="ExternalInput")
    W2d = nc.dram_tensor("W2", [H1, H2], MM_DT, kind="ExternalInput")
    W3d = nc.dram_tensor("W3", [H2, OUTP], MM_DT, kind="ExternalInput")
    b1d = nc.dram_tensor("b1", [128, 7], F32, kind="ExternalInput")
    b2d = nc.dram_tensor("b2", [128, 6], F32, kind="ExternalInput")
    b3d = nc.dram_tensor("b3", [1, OUTP], MM_DT, kind="ExternalInput")
    codes_d = nc.dram_tensor("codes", [128, NSUB], mybir.dt.int32,
                             kind="ExternalOutput")
    gaps_d = nc.dram_tensor("gaps", [128, NSUB], F32, kind="ExternalOutput")

    KC1 = _ceil(DIM, 128)   # 8 (exact)
    KC2 = _ceil(H1, 128)    # 7, last 126
    KC3 = _ceil(H2, 128)    # 6, last 123
    MT1 = _ceil(H1, 128)    # 7, last 126
    MT2 = _ceil(H2, 128)    # 6, last 123

    LR = mybir.ActivationFunctionType.Lrelu

    with tile.TileContext(nc) as tc:
        with (
            tc.tile_pool(name="wpool", bufs=1) as wp,
            tc.tile_pool(name="xpool", bufs=2) as xp,
            tc.tile_pool(name="hpool", bufs=2) as hp,
            tc.tile_pool(name="spool", bufs=3) as sp,
            tc.tile_pool(name="cpool", bufs=1) as cp,
            tc.tile_pool(name="ps12", bufs=3, space="PSUM") as ps12,
            tc.tile_pool(name="ps3", bufs=3, space="PSUM") as ps3,
        ):
            # ---- weights / biases (loaded once) ----
            w1 = wp.tile([128, KC1, H1], MM_DT)
            nc.sync.dma_start(
                out=w1, in_=W1d[:].rearrange("(kc p) m -> p kc m", p=128))
            w2 = wp.tile([128, KC2, H2], MM_DT)
            nc.sync.dma_start(
                out=w2[:, 0:6, :],
                in_=W2d[0:768, :].rearrange("(kc p) m -> p kc m", p=128))
            nc.sync.dma_start(
                out=w2[0:126, 6, :], in_=W2d[768:894, :])
            w3 = wp.tile([128, KC3, OUTP], MM_DT)
            nc.sync.dma_start(
                out=w3[:, 0:5, :],
                in_=W3d[0:640, :].rearrange("(kc p) m -> p kc m", p=128))
            nc.sync.dma_start(
                out=w3[0:123, 5, :], in_=W3d[640:763, :])
            b1 = wp.tile([128, 7], F32)
            nc.sync.dma_start(out=b1, in_=b1d[:])
            b2 = wp.tile([128, 6], F32)
            nc.sync.dma_start(out=b2, in_=b2d[:])
            b3 = wp.tile([1, OUTP], MM_DT)
            nc.sync.dma_start(out=b3, in_=b3d[:])
            ones_f = wp.tile([1, 128], F32)
            nc.vector.memset(ones_f, 1.0)
            ones = wp.tile([1, 128], MM_DT)
            nc.vector.tensor_copy(ones, ones_f)

            codes_sb = cp.tile([128, NSUB], mybir.dt.int32)
            gaps_sb = cp.tile([128, NSUB], F32)

            for c in range(NCHUNK):
                xs = xp.tile([128, KC1, CH], MM_DT, tag="xslab")
                nc.sync.dma_start(
                    out=xs,
                    in_=xT[:, c * CH:(c + 1) * CH].rearrange(
                        "(kc p) t -> p kc t", p=128))

                # ---- layer 1: h1T[m*128+p, t] ----
                h1t = hp.tile([128, KC2, CH], MM_DT, tag="h1t")
                for mt in range(MT1):
                    m0 = mt * 128
                    mw = min(128, H1 - m0)
                    pt = ps12.tile([128, CH], F32, tag="pmm")
                    for kc in range(KC1):
                        nc.tensor.matmul(
                            pt[:mw, :], w1[:, kc, m0:m0 + mw], xs[:, kc, :],
                            start=(kc == 0), stop=(kc == KC1 - 1))
                    nc.scalar.activation(
                        h1t[:mw, mt, :], pt[:mw, :], LR,
                        bias=b1[:mw, mt:mt + 1], scale=1.0, alpha=0.01)

                # ---- layer 2 ----
                h2t = hp.tile([128, KC3, CH], MM_DT, tag="h2t")
                for mt in range(MT2):
                    m0 = mt * 128
                    mw = min(128, H2 - m0)
                    pt = ps12.tile([128, CH], F32, tag="pmm")
                    for kc in range(KC2):
                        kw = min(128, H1 - kc * 128)
                        nc.tensor.matmul(
                            pt[:mw, :], w2[:kw, kc, m0:m0 + mw],
                            h1t[:kw, kc, :],
                            start=(kc == 0), stop=(kc == KC2 - 1))
                    nc.scalar.activation(
                        h2t[:mw, mt, :], pt[:mw, :], LR,
                        bias=b2[:mw, mt:mt + 1], scale=1.0, alpha=0.01)

                # ---- layer 3 + argmax: per 128-token subtile ----
                for s in range(4):
                    t0 = s * 128
                    pl = ps3.tile([128, OUTP], F32, tag="plog")
                    nc.tensor.matmul(pl, ones, b3, start=True, stop=False)
                    for kc in range(KC3):
                        kw = min(128, H2 - kc * 128)
                        nc.tensor.matmul(
                            pl, h2t[:kw, kc, t0:t0 + 128], w3[:kw, kc, :],
                            start=False, stop=(kc == KC3 - 1))
                    logit = sp.tile([128, OUTP], F32, tag="logit")
                    nc.scalar.copy(logit, pl)
                    mx8 = sp.tile([128, 8], F32, tag="mx8")
                    ix8 = sp.tile([128, 8], mybir.dt.uint32, tag="ix8")
                    nc.vector.max(mx8, logit)
                    nc.vector.max_index(ix8, mx8, logit)
                    col = c * 4 + s
                    nc.vector.tensor_copy(
                        codes_sb.bitcast(mybir.dt.uint32)[:, col:col + 1],
                        ix8[:, 0:1])
                    nc.vector.tensor_sub(
                        gaps_sb[:, col:col + 1], mx8[:, 0:1], mx8[:, 1:2])

            nc.sync.dma_start(out=codes_d[:], in_=codes_sb)
            nc.sync.dma_start(out=gaps_d[:], in_=gaps_sb)

    nc.finalize()
    return nc


def _forward_fill_exact(code_flat: np.ndarray) -> np.ndarray:
    """Exact equivalent of the reference jax while-loop fill."""
    n = code_flat.shape[0]
    mask = code_flat == VOCAB
    if not mask.any():
        return code_flat
    if mask.all():
        return code_flat
    idx = np.where(~mask, np.arange(n), -1)
    fill = np.maximum.accumulate(idx)
    # wrap-around: positions before first non-stop take the last non-stop
    last = np.max(idx)
    dist = np.arange(n) - fill
    wrapped = fill < 0
    fill = np.where(wrapped, last, fill)
    dist = np.where(wrapped, np.arange(n) + (n - last), dist)
    out = code_flat[fill]
    # faithful MAX_ITERS cap: stops further than MAX_ITERS remain
    out = np.where(mask & (dist > MAX_ITERS), VOCAB, out)
    out = np.where(mask, out, code_flat)
    return out.astype(np.int32)


def kernel(x, W1, b1, W2, b2, W3, b3):
    x = np.asarray(x, dtype=np.float32)
    W1 = np.ascontiguousarray(np.asarray(W1, dtype=np.float32))
    W2 = np.ascontiguousarray(np.asarray(W2, dtype=np.float32))
    W3 = np.ascontiguousarray(np.asarray(W3, dtype=np.float32))
    b1 = np.asarray(b1, dtype=np.float32)
    b2 = np.asarray(b2, dtype=np.float32)
    b3 = np.asarray(b3, dtype=np.float32)

    if "nc" not in _CACHE:
        _CACHE["nc"] = build_kernel()
    nc = _CACHE["nc"]

    b1p = np.zeros((7 * 128,), np.float32)
    b1p[:H1] = b1
    b1p = np.ascontiguousarray(b1p.reshape(7, 128).T)
    b2p = np.zeros((6 * 128,), np.float32)
    b2p[:H2] = b2
    b2p = np.ascontiguousarray(b2p.reshape(6, 128).T)
    b3p = np.full((1, OUTP), -1e30, np.float32)
    b3p[0, :OUT] = b3
    W3p = np.zeros((H2, OUTP), np.float32)
    W3p[:, :OUT] = W3

    in_maps = []
    for i in range(NCORES):
        xs = x[2 * i:2 * i + 2].reshape(REAL_TOK, DIM)
        xp = np.zeros((TOK, DIM), np.float32)
        xp[:REAL_TOK] = xs
        xTp = np.ascontiguousarray(xp.T)
        in_maps.append({
            "xT": xTp, "W1": W1, "W2": W2, "W3": W3p,
            "b1": b1p, "b2": b2p, "b3": b3p,
        })

    _CACHE["in_maps"] = in_maps
    try:
        res = run_bass_kernel_spmd(nc, in_maps, core_ids=list(range(NCORES)))
    except Exception:
        # transient NRT device wedge: one retry usually recovers
        res = run_bass_kernel_spmd(nc, in_maps, core_ids=list(range(NCORES)))

    parts, gparts = [], []
    for i in range(NCORES):
        codes = res.results[i]["codes"]          # [128, 32]
        parts.append(codes.T.reshape(-1)[:REAL_TOK])   # token t = s*128+p
        gparts.append(res.results[i]["gaps"].T.reshape(-1)[:REAL_TOK])
    code = np.concatenate(parts).astype(np.int32)   # [32000]
    gap = np.concatenate(gparts).astype(np.float32)

    # fp32r argmax can flip near-ties; recompute uncertain tokens exactly
    unc = np.flatnonzero(gap < 1e-2)
    if unc.size:
        xf = x.reshape(-1, DIM)[unc].astype(np.float32)
        h = xf @ W1 + b1
        h = np.where(h >= 0, h, np.float32(0.01) * h).astype(np.float32)
        h = h @ W2 + b2
        h = np.where(h >= 0, h, np.float32(0.01) * h).astype(np.float32)
        lg = h @ W3 + b3
        code[unc] = np.argmax(lg, axis=-1).astype(np.int32)

    code = _forward_fill_exact(code)
    return code.reshape(B, T)


# revision 12
# speedup vs baseline: 1.7434x; 1.7434x over previous
"""Trainium2 Bass kernel for nn_LinearQuantizerModel.

MLP 1024->894->763->501 (leaky_relu 0.01) + argmax over classes + exact
forward-fill of stop tokens (==500) done on host.

Sharding: data-parallel over batch B=16 across 8 cores (2 batches/core),
weights replicated. Per core 4000 tokens padded to 4096 = 8 chunks x 512.

Device layout: features on partitions, tokens on free axis. x is
transposed on host so DMA loads are contiguous. Matmuls run in fp32r
(full-rate PE). Layer-3 flips orientation (stationary = H2T token tile,
moving = W3) so logits land [tokens, 501] in PSUM for vector argmax.
"""

import numpy as np

import concourse.bass as bass
import concourse.mybir as mybir
import concourse.tile as tile
from concourse import bacc
from concourse.bass_utils import run_bass_kernel_spmd

B, T, DIM, H1, H2, OUT = 16, 2000, 1024, 894, 763, 501
OUTP = 512  # class dim padded for fp32r ISA (even/aligned free dim)
VOCAB = 500
MAX_ITERS = 10000
NCORES = 8
TOK = 4096          # padded tokens per core (4000 real)
REAL_TOK = 4000
NCHUNK = 8
CH = 512            # tokens per chunk
NSUB = TOK // 128   # 32 code columns

F32 = mybir.dt.float32
F32R = mybir.dt.float32r
MM_DT = F32R        # fp32r: full-rate PE, ~1e-4 matmul precision

_CACHE = {}


def _ceil(a, b):
    return (a + b - 1) // b


def build_kernel():
    nc = bacc.Bacc(target_bir_lowering=False)

    xT = nc.dram_tensor("xT", [DIM, TOK], MM_DT, kind="ExternalInput")
    W1d = nc.dram_tensor("W1", [DIM, H1], MM_DT, kind="ExternalInput")
    W2d = nc.dram_tensor("W2", [H1, H2], MM_DT, kind="ExternalInput")
    W3d = nc.dram_tensor("W3", [H2, OUTP], MM_DT, kind="ExternalInput")
    b1d = nc.dram_tensor("b1", [128, 7], F32, kind="ExternalInput")
    b2d = nc.dram_tensor("b2", [128, 6], F32, kind="ExternalInput")
    b3d = nc.dram_tensor("b3", [1, OUTP], MM_DT, kind="ExternalInput")
    codes_d = nc.dram_tensor("codes", [128, NSUB], mybir.dt.int32,
                             kind="ExternalOutput")
    gaps_d = nc.dram_tensor("gaps", [128, NSUB], F32, kind="ExternalOutput")

    KC1 = _ceil(DIM, 128)   # 8 (exact)
    KC2 = _ceil(H1, 128)    # 7, last 126
    KC3 = _ceil(H2, 128)    # 6, last 123
    MT1 = _ceil(H1, 128)    # 7, last 126
    MT2 = _ceil(H2, 128)    # 6, last 123

    LR = mybir.ActivationFunctionType.Lrelu

    with tile.TileContext(nc) as tc:
        with (
            tc.tile_pool(name="wpool", bufs=1) as wp,
            tc.tile_pool(name="xpool", bufs=3) as xp,
            tc.tile_pool(name="hpool", bufs=2) as hp,
            tc.tile_pool(name="spool", bufs=3) as sp,
            tc.tile_pool(name="cpool", bufs=1) as cp,
            tc.tile_pool(name="ps12", bufs=4, space="PSUM") as ps12,
            tc.tile_pool(name="ps3", bufs=3, space="PSUM") as ps3,
        ):
            # ---- weights / biases (loaded once) ----
            w1 = wp.tile([128, KC1, H1], MM_DT)
            nc.sync.dma_start(
                out=w1, in_=W1d[:].rearrange("(kc p) m -> p kc m", p=128))
            w2 = wp.tile([128, KC2, H2], MM_DT)
            nc.sync.dma_start(
                out=w2[:, 0:6, :],
                in_=W2d[0:768, :].rearrange("(kc p) m -> p kc m", p=128))
            nc.sync.dma_start(
                out=w2[0:126, 6, :], in_=W2d[768:894, :])
            w3 = wp.tile([128, KC3, OUTP], MM_DT)
            nc.sync.dma_start(
                out=w3[:, 0:5, :],
                in_=W3d[0:640, :].rearrange("(kc p) m -> p kc m", p=128))
            nc.sync.dma_start(
                out=w3[0:123, 5, :], in_=W3d[640:763, :])
            b1 = wp.tile([128, 7], F32)
            nc.sync.dma_start(out=b1, in_=b1d[:])
            b2 = wp.tile([128, 6], F32)
            nc.sync.dma_start(out=b2, in_=b2d[:])
            b3 = wp.tile([1, OUTP], MM_DT)
            nc.sync.dma_start(out=b3, in_=b3d[:])
            ones_f = wp.tile([1, 128], F32)
            nc.vector.memset(ones_f, 1.0)
            ones = wp.tile([1, 128], MM_DT)
            nc.vector.tensor_copy(ones, ones_f)

            codes_sb = cp.tile([128, NSUB], mybir.dt.int32)
            gaps_sb = cp.tile([128, NSUB], F32)

            for c in range(NCHUNK):
                xs = xp.tile([128, KC1, CH], MM_DT, tag="xslab")
                nc.sync.dma_start(
                    out=xs,
                    in_=xT[:, c * CH:(c + 1) * CH].rearrange(
                        "(kc p) t -> p kc t", p=128))

                # ---- layer 1: h1T[m*128+p, t] ----
                h1t = hp.tile([128, KC2, CH], MM_DT, tag="h1t")
                for mt in range(MT1):
                    m0 = mt * 128
                    mw = min(128, H1 - m0)
                    pt = ps12.tile([128, CH], F32, tag="pmm")
                    for kc in range(KC1):
                        nc.tensor.matmul(
                            pt[:mw, :], w1[:, kc, m0:m0 + mw], xs[:, kc, :],
                            start=(kc == 0), stop=(kc == KC1 - 1))
                    nc.scalar.activation(
                        h1t[:mw, mt, :], pt[:mw, :], LR,
                        bias=b1[:mw, mt:mt + 1], scale=1.0, alpha=0.01)

                # ---- layer 2 ----
                h2t = hp.tile([128, KC3, CH], MM_DT, tag="h2t")
                for mt in range(MT2):
                    m0 = mt * 128
                    mw = min(128, H2 - m0)
                    pt = ps12.tile([128, CH], F32, tag="pmm")
                    for kc in range(KC2):
                        kw = min(128, H1 - kc * 128)
                        nc.tensor.matmul(
                            pt[:mw, :], w2[:kw, kc, m0:m0 + mw],
                            h1t[:kw, kc, :],
                            start=(kc == 0), stop=(kc == KC2 - 1))
                    nc.scalar.activation(
                        h2t[:mw, mt, :], pt[:mw, :], LR,
                        bias=b2[:mw, mt:mt + 1], scale=1.0, alpha=0.01)

                # ---- layer 3 + argmax: per 128-token subtile ----
                for s in range(4):
                    t0 = s * 128
                    pl = ps3.tile([128, OUTP], F32, tag="plog")
                    nc.tensor.matmul(pl, ones, b3, start=True, stop=False)
                    for kc in range(KC3):
                        kw = min(128, H2 - kc * 128)
                        nc.tensor.matmul(
                            pl, h2t[:kw, kc, t0:t0 + 128], w3[:kw, kc, :],
                            start=False, stop=(kc == KC3 - 1))
                    logit = sp.tile([128, OUTP], F32, tag="logit")
                    nc.scalar.copy(logit, pl)
                    mx8 = sp.tile([128, 8], F32, tag="mx8")
                    ix8 = sp.tile([128, 8], mybir.dt.uint32, tag="ix8")
                    nc.vector.max(mx8, logit)
                    nc.vector.max_index(ix8, mx8, logit)
                    col = c * 4 + s
                    nc.vector.tensor_copy(
                        codes_sb.bitcast(mybir.dt.uint32)[:, col:col + 1],
                        ix8[:, 0:1])
                    nc.vector.tensor_sub(
                        gaps_sb[:, col:col + 1], mx8[:, 0:1], mx8[:, 1:2])

            nc.sync.dma_start(out=codes_d[:], in_=codes_sb)
            nc.sync.dma_start(out=gaps_d[:], in_=gaps_sb)

    nc.finalize()
    return nc


def _forward_fill_exact(code_flat: np.ndarray) -> np.ndarray:
    """Exact equivalent of the reference jax while-loop fill."""
    n = code_flat.shape[0]
    mask = code_flat == VOCAB
    if not mask.any():
        return code_flat
    if mask.all():
        return code_flat
    idx = np.where(~mask, np.arange(n), -1)
    fill = np.maximum.accumulate(idx)
    # wrap-around: positions before first non-stop take the last non-stop
    last = np.max(idx)
    dist = np.arange(n) - fill
    wrapped = fill < 0
    fill = np.where(wrapped, last, fill)
    dist = np.where(wrapped, np.arange(n) + (n - last), dist)
    out = code_flat[fill]
    # faithful MAX_ITERS cap: stops further than MAX_ITERS remain
    out = np.where(mask & (dist > MAX_ITERS), VOCAB, out)
    out = np.where(mask, out, code_flat)
    return out.astype(np.int32)


def kernel(x, W1, b1, W2, b2, W3, b3):
    x = np.asarray(x, dtype=np.float32)
    W1 = np.ascontiguousarray(np.asarray(W1, dtype=np.float32))
    W2 = np.ascontiguousarray(np.asarray(W2, dtype=np.float32))
    W3 = np.ascontiguousarray(np.asarray(W3, dtype=np.float32))
    b1 = np.asarray(b1, dtype=np.float32)
    b2 = np.asarray(b2, dtype=np.float32)
    b3 = np.asarray(b3, dtype=np.float32)

    if "nc" not in _CACHE:
        _CACHE["nc"] = build_kernel()
    nc = _CACHE["nc"]

    b1p = np.zeros((7 * 128,), np.float32)
    b1p[:H1] = b1
    b1p = np.ascontiguousarray(b1p.reshape(7, 128).T)
    b2p = np.zeros((6 * 128,), np.float32)
    b2p[:H2] = b2
    b2p = np.ascontiguousarray(b2p.reshape(6, 128).T)
    b3p = np.full((1, OUTP), -1e30, np.float32)
    b3p[0, :OUT] = b3
    W3p = np.zeros((H2, OUTP), np.float32)
    W3p[:, :OUT] = W3

    # one vectorized pad+transpose pass for all shards
    xa = np.zeros((NCORES, TOK, DIM), np.float32)
    xa[:, :REAL_TOK] = x.reshape(NCORES, REAL_TOK, DIM)
    xTa = np.ascontiguousarray(xa.transpose(0, 2, 1))
    in_maps = []
    for i in range(NCORES):
        in_maps.append({
            "xT": xTa[i], "W1": W1, "W2": W2, "W3": W3p,
            "b1": b1p, "b2": b2p, "b3": b3p,
        })

    _CACHE["in_maps"] = in_maps
    try:
        res = run_bass_kernel_spmd(nc, in_maps, core_ids=list(range(NCORES)))
    except Exception:
        # transient NRT device wedge: one retry usually recovers
        res = run_bass_kernel_spmd(nc, in_maps, core_ids=list(range(NCORES)))

    parts, gparts = [], []
    for i in range(NCORES):
        codes = res.results[i]["codes"]          # [128, 32]
        parts.append(codes.T.reshape(-1)[:REAL_TOK])   # token t = s*128+p
        gparts.append(res.results[i]["gaps"].T.reshape(-1)[:REAL_TOK])
    code = np.concatenate(parts).astype(np.int32)   # [32000]
    gap = np.concatenate(gparts).astype(np.float32)

    # fp32r argmax can flip near-ties; recompute uncertain tokens exactly
    unc = np.flatnonzero(gap < 1e-2)
    if unc.size:
        xf = x.reshape(-1, DIM)[unc].astype(np.float32)
        h = xf @ W1 + b1
        h = np.where(h >= 0, h, np.float32(0.01) * h).astype(np.float32)
        h = h @ W2 + b2
        h = np.where(h >= 0, h, np.float32(0.01) * h).astype(np.float32)
        lg = h @ W3 + b3
        code[unc] = np.argmax(lg, axis=-1).astype(np.int32)

    code = _forward_fill_exact(code)
    return code.reshape(B, T)
